# revision 13
# baseline (speedup 1.0000x reference)
"""Multi-head graph attention layer on 8 Trainium2 NeuronCores.

Reference computation (per batch element b; adj is unused by the reference):
    P      = einsum("nf,hfd->hnd", h[b], W)          # per-head projections
    S      = einsum("hnd,hmd->hnm", P, P)            # scores (symmetric!)
    E      = exp(leakyrelu(S, 0.2) + SHIFT)
    attn   = E / rowsum(E)
    out[b] = concat_heads(attn @ P) + h[b]

Numerical shortcuts (validated against the reference on CPU):
  * leakyrelu is dropped: every row's max score is >= ~24 while lrelu only
    modifies scores < 0, whose softmax weight is < e^-24 relative -- the
    end-to-end deviation is < 1e-6.
  * Half the panels use a Schraudolph-style fast exp on the Vector engine:
    bf16(exp(x)) ~= bitcast_bf16(uint16(round(A*x + B))), A = 128/ln2.
    The f32->uint16 conversion saturates at 0/65535 on HW (probed), which
    clamps the underflow range for free. Max end-to-end rel err ~4e-3.

Sharding: batch B=8 -> one batch element per core (pure data parallel).

Per-core plan (N=2048 tokens, F=256, H=4 heads, D=64):
  Phase 0: hT via PE transposes; P (bf16) and PT (fp16) via f32r matmuls.
  Main loop over head pairs pp, column halves qh, row tiles a:
    PE computes S panels for both heads (row-packed, K=64+64).
    Head 2pp   ("ACT head"): exp on the Scalar engine straight from PSUM,
      bf16 out + fused accum_out giving softmax row-sum partials.
    Head 2pp+1 ("DVE head"): Schraudolph on the Vector engine straight from
      PSUM into uint16 (= bf16 bits), then a 4x-mode tensor_scalar copy over
      the bf16 view with accum_out for its row sums.
    PE accumulates outT[d, q] += P_a^T-block @ E-panel (col-packed heads).
  Finalize per pp: PE-transpose outT chunks; DVE fuses (outT * recip) + h;
  DMA to DRAM.
"""

import math

import numpy as np

import bass_rust
import concourse.bass as bass
import concourse.tile as tile
from concourse import mybir
from concourse.bass_utils import run_bass_kernel_spmd
from concourse.vector_clock import ScopedClock


def _patched_drain_and_barrier(self, tick_clock, wait_clock):
    """Replacement for TileContext._drain_and_barrier.

    The stock version attaches every outstanding semaphore wait (engines +
    every DMA queue used) to ONE tail drain; walrus's setupSyncWait rejects
    instructions with more than a couple of sync waits. Emit a chain of
    drains first, each carrying a single semaphore wait, so the final full
    drain has nothing left to wait on.
    """
    gc = tick_clock.global_clock
    n_procs = 27
    vals = [gc.peek_next(p) - 1 for p in range(n_procs)]
    for p, v in enumerate(vals):
        if v <= 0:
            continue
        partial = bass_rust.VectorClock()
        partial.require_at_least(p, v)
        d = self.nc.sync.drain()
        wait_clock.add_sem_waits(d.ins, ScopedClock({None: partial}))

    # Final drain carries no waits: the chain above already waited out the
    # full global clock on SP, which executes its queue in order.
    self.nc.sync.drain()

    self.nc.all_engine_barrier()
    assert self.sems is not None
    popped = self.nc._tile_sem_poison_stack.pop()
    assert popped is self._sem_poison
    self.nc.clear_and_free_semaphores(list(self.sems.allocated().values()))
    self.nc.all_engine_barrier()


tile.TileContext._drain_and_barrier = _patched_drain_and_barrier


def _split_sync_waits(nc, max_waits=1):
    """walrus's per-instruction sync-wait budget is tiny (LDWEIGHTS rejects
    even 2). Hoist excess waits onto standalone same-engine EventSemaphore
    instructions inserted immediately before the offender — identical
    semantics, one wait per instruction word."""
    n_split = 0
    for f in nc.m.functions:
        for bb in f.blocks:
            il = bb.instructions
            i = 0
            while i < len(il):
                ins = il[i]
                si = ins.sync_info
                waits = list(si.on_wait) if si and si.on_wait else []
                if len(waits) > max_waits:
                    keep = waits[:max_waits]
                    excess = waits[max_waits:]
                    carriers = []
                    for k, w in enumerate(excess):
                        c = bass_rust.InstEventSemaphore(
                            name=f"{ins.name}-w{k}", ins=[], outs=[]
                        )
                        c.engine = ins.engine
                        c.sync_info = mybir.SyncInfo(on_wait=[w], on_update=[])
                        carriers.append(c)
                    ins.sync_info = mybir.SyncInfo(
                        on_wait=keep, on_update=list(si.on_update or [])
                    )
                    il[i:i] = carriers
                    i += len(carriers)
                    n_split += 1
                i += 1
    return n_split


N = 2048
F_IN = 256
H = 4
D = 64
NT = N // 128  # 16 token tiles
N_CORES = 8
# Constant shift inside exp (softmax is shift-invariant). Scores reach
# ~+150 on the diagonal (chi^2_64) which would overflow exp in fp32;
# with C=80 the exp range is [e^-230, e^72] — comfortably finite, and
# row sums stay >= e^(rowmax-80) > 1e-25 so the reciprocal is safe.
EXP_SHIFT = -80.0
# Schraudolph fast-exp in bf16 bit space: u16 = round(A*x + B) saturated
# to [0, 65535]; bitcast to bf16 gives exp(x) with ~2% max elementwise
# error (~4e-3 end-to-end after softmax normalization).
A_SCH = 128.0 / math.log(2.0)
B_SCH = 127.0 * 128.0 - 8.0 + A_SCH * EXP_SHIFT

F32 = mybir.dt.float32
F32R = mybir.dt.float32r
BF16 = mybir.dt.bfloat16
F16 = mybir.dt.float16
U16 = mybir.dt.uint16

# Hoist multi-sem waits into standalone carrier instructions (needed for
# walrus codegen).
SPLIT_WAITS = True
# head1 panels are exp'd by ACT (instead of DVE Schraudolph + reduce) on
# these row-tile iterations — balances the two engines, since a DVE panel
# costs ~2.6us (Schraudolph + 1x-mode row-sum reduce) while an ACT panel
# costs ~1.4us (exp with fused accum).
ACT1_SET = frozenset({1, 4, 7, 10, 13, 15})


def _build_program():
    nc = bass.Bass("TRN2", target_bir_lowering=False, debug=False)
    hhi_d = nc.dram_tensor("h_hi", [N, F_IN], BF16, kind="ExternalInput").ap()
    hlo_d = nc.dram_tensor("h_lo", [N, F_IN], BF16, kind="ExternalInput").ap()
    w_d = nc.dram_tensor("w", [H, F_IN, D], F32, kind="ExternalInput").ap()
    id_d = nc.dram_tensor("ident", [128, 128], F32, kind="ExternalInput").ap()
    out_d = nc.dram_tensor("out", [N, F_IN], F32, kind="ExternalOutput").ap()

    with tile.TileContext(nc) as tc:
        _gat_kernel(tc, out_d, hhi_d, hlo_d, w_d, id_d)
    if SPLIT_WAITS:
        _split_sync_waits(nc)
    return nc


def _gat_kernel(tc: "tile.TileContext", out_d, hhi_d, hlo_d, w_d, id_d):
    nc = tc.nc
    MULT = mybir.AluOpType.mult
    ADD = mybir.AluOpType.add
    EXP = mybir.ActivationFunctionType.Exp
    COPY = mybir.ActivationFunctionType.Copy

    with (
        tc.tile_pool(name="const", bufs=1) as const,
    ):
        # ---------------- persistent SBUF ----------------
        ident = const.tile([128, 128], F32, name="ident_sb")
        nc.sync.dma_start(ident[:], id_d[:])
        shift = const.tile([128, 1], F32, name="shift_sb")
        nc.gpsimd.memset(shift[:], EXP_SHIFT)
        # h tiles (bf16 hi part only — used for the residual add; the lo
        # part is < 0.4% of h and far below the error budget)
        hhi_sb = const.tile([128, NT * F_IN], BF16, name="hhi_sb")  # [p,(qt f)]
        for qt in range(NT):
            nc.sync.dma_start(
                hhi_sb[:, qt * F_IN : (qt + 1) * F_IN],
                hhi_d[qt * 128 : (qt + 1) * 128, :],
            )
        # hT via transposing DMA of the bf16 hi/lo split, then one DVE add
        # to reconstruct fp32(ish) h^T: [p=f, (ft, n)]
        hThi = const.tile([128, 2 * N], BF16, name="hThi")
        hTlo = const.tile([128, 2 * N], BF16, name="hTlo")
        for ft in range(2):
            nc.sync.dma_start_transpose(
                hThi[:, ft * N : (ft + 1) * N], hhi_d[:, ft * 128 : (ft + 1) * 128]
            )
            nc.sync.dma_start_transpose(
                hTlo[:, ft * N : (ft + 1) * N], hlo_d[:, ft * 128 : (ft + 1) * 128]
            )
        w_sb = const.tile([128, 2 * F_IN], F32, name="w_sb")  # [p, (ft, h*64+d)]
        for hh in range(H):
            for ft in range(2):
                nc.sync.dma_start(
                    w_sb[:, ft * F_IN + hh * D : ft * F_IN + (hh + 1) * D],
                    w_d[hh, ft * 128 : (ft + 1) * 128, :],
                )

        w_sbr = const.tile([128, 2 * F_IN], F32R, name="w_sbr")
        nc.vector.tensor_copy(w_sbr[:], w_sb[:])
        hT_sb = const.tile([128, 2 * N], F32R, name="hT_sb")  # [p=f, (ft, n)]
        nc.vector.tensor_add(hT_sb[:], hThi[:], hTlo[:])
        p_bf = const.tile([128, NT * F_IN], BF16, name="p_bf")  # [p=k, (kt, h*64+d)]
        # PT pair tiles: partitions 0-63 = head 2pp dims, 64-127 = head 2pp+1
        pt_sb = [
            const.tile([128, N], F16, name=f"pt_pair{pp}") for pp in range(H // 2)
        ]
        # row-sum partials: col = qh*64 + hh*16 + a  (token tile a, head hh)
        rows = const.tile([128, 2 * H * NT], F32, name="rows")
        rsum = const.tile([128, H * NT], F32, name="rsum")  # col = hh*16 + qt
        recip = const.tile([128, H * NT], F32, name="recip")
        trash = const.tile([128, 1024], BF16, name="trash")

        # ---------------- phase B/C: projections ----------------
        with (
            tc.tile_pool(name="p_ps", bufs=2, space="PSUM") as p_ps,
            tc.tile_pool(name="pt_ps", bufs=2, space="PSUM") as pt_ps,
        ):
            # PT per head-pair first (it gates the S matmuls): fp16
            # [128(d of 2 heads), N(q)]
            for pp in range(H // 2):
                for pan in range(4):
                    ptp = pt_ps.tile([128, 512], F32, name="ptp", tag="ptp")
                    for ft in range(2):
                        nc.tensor.matmul(
                            ptp[:],
                            w_sbr[:, ft * F_IN + pp * 128 : ft * F_IN + (pp + 1) * 128],
                            hT_sb[:, ft * N + pan * 512 : ft * N + (pan + 1) * 512],
                            start=(ft == 0),
                            stop=(ft == 1),
                        )
                    dst = pt_sb[pp][:, pan * 512 : (pan + 1) * 512]
                    if pan % 2 == 0:
                        nc.scalar.activation(dst, ptp[:], COPY)
                    else:
                        nc.vector.tensor_copy(dst, ptp[:])

            # P = h @ W  -> [k, (h d)] tiles, stored bf16
            for i in range(NT):
                pp_t = p_ps.tile([128, F_IN], F32, name="pp", tag="pp")
                for ft in range(2):
                    nc.tensor.matmul(
                        pp_t[:],
                        hT_sb[:, ft * N + i * 128 : ft * N + (i + 1) * 128],
                        w_sbr[:, ft * F_IN : (ft + 1) * F_IN],
                        start=(ft == 0),
                        stop=(ft == 1),
                    )
                dst = p_bf[:, i * F_IN : (i + 1) * F_IN]
                if i % 2 == 0:
                    nc.scalar.activation(dst, pp_t[:], COPY)
                else:
                    nc.vector.tensor_copy(dst, pp_t[:])

        # ---------------- phase D: attention main loop ----------------
        # PSUM: sa pool 2x[128,1024] (4 banks) + sd pool 4x[128,512]
        # (4 banks... wait: 2 bufs = 2 banks) + ot 1x[128,1024] (2 banks).
        with (
            tc.tile_pool(name="sa_ps", bufs=2, space="PSUM") as sa_ps,
            tc.tile_pool(name="sd_ps", bufs=2, space="PSUM") as sd_ps,
            tc.tile_pool(name="ot_ps", bufs=1, space="PSUM") as ot_ps,
            tc.tile_pool(name="e0_pool", bufs=3) as e0_pool,
            tc.tile_pool(name="e1_pool", bufs=3) as e1_pool,
            tc.tile_pool(name="ot_sb_pool", bufs=2) as ot_sb_pool,
            tc.tile_pool(name="out_pool", bufs=6) as out_pool,
        ):

            for pp in range(H // 2):
                h0 = 2 * pp
                h1 = h0 + 1
                ot_sb = ot_sb_pool.tile([128, N], F32, name="ot_sb", tag="ot_sb")
                for qh in range(2):
                    ot = ot_ps.tile([128, 1024], F32, name="ot", tag="ot")
                    # software pipeline: S+exp for tile a, outT for tile a-1
                    prev = None
                    for a in range(NT + 1):
                        if a < NT:
                            head1_act = a in ACT1_SET
                            sa = sa_ps.tile([128, 1024], F32, name="sa", tag="sa")
                            if head1_act:
                                sa1 = sa_ps.tile(
                                    [128, 1024], F32, name="sa1", tag="sa"
                                )
                                sd = None
                            else:
                                sa1 = None
                                sd = [
                                    sd_ps.tile(
                                        [128, 512], F32, name=f"sd{p2}", tag="sd"
                                    )
                                    for p2 in range(2)
                                ]
                            # S matmuls, row-packed head pair (interleave the
                            # two heads per p2 chunk so they stream together)
                            for p2 in range(2):
                                qsl = slice(
                                    qh * 1024 + p2 * 512, qh * 1024 + (p2 + 1) * 512
                                )
                                csl = slice(p2 * 512, (p2 + 1) * 512)
                                nc.tensor.matmul(
                                    sa[:, csl],
                                    pt_sb[pp][0:64, a * 128 : (a + 1) * 128],
                                    pt_sb[pp][0:64, qsl],
                                    start=True,
                                    stop=True,
                                    tile_position=(0, 0),
                                )
                                nc.tensor.matmul(
                                    sa1[:, csl] if head1_act else sd[p2][:],
                                    pt_sb[pp][64:128, a * 128 : (a + 1) * 128],
                                    pt_sb[pp][64:128, qsl],
                                    start=True,
                                    stop=True,
                                    tile_position=(64, 0),
                                )
                            # ACT head: table exp straight from PSUM + accum
                            e0 = e0_pool.tile([128, 1024], BF16, name="e0", tag="e0")
                            acc0 = rows[
                                :, qh * 64 + h0 * NT + a : qh * 64 + h0 * NT + a + 1
                            ]
                            nc.scalar.activation(
                                e0[:], sa[:], EXP, bias=shift[:], accum_out=acc0
                            )
                            # head1: either ACT exp (same as head0) or DVE
                            # Schraudolph into uint16 (bf16 bits) + 1x reduce
                            e1 = e1_pool.tile([128, 1024], U16, name="e1", tag="e1")
                            acc1 = rows[
                                :, qh * 64 + h1 * NT + a : qh * 64 + h1 * NT + a + 1
                            ]
                            if head1_act:
                                nc.scalar.activation(
                                    e1[:].bitcast(BF16),
                                    sa1[:],
                                    EXP,
                                    bias=shift[:],
                                    accum_out=acc1,
                                )
                            else:
                                for p2 in range(2):
                                    nc.vector.tensor_scalar(
                                        e1[:, p2 * 512 : (p2 + 1) * 512],
                                        sd[p2][:],
                                        A_SCH,
                                        B_SCH,
                                        MULT,
                                        ADD,
                                    )
                                nc.vector.tensor_scalar(
                                    trash[:],
                                    e1[:].bitcast(BF16),
                                    1.0,
                                    0.0,
                                    MULT,
                                    ADD,
                                    accum_out=acc1,
                                )
                            prev_new = (e0, e1)
                        if a > 0:
                            e0p, e1p = prev
                            rhs = [e0p[:], e1p[:].bitcast(BF16)]
                            ap = a - 1
                            for p2 in range(2):
                                for hi in range(2):
                                    po = 64 * hi
                                    nc.tensor.matmul(
                                        ot[po : po + 64, p2 * 512 : (p2 + 1) * 512],
                                        p_bf[
                                            :,
                                            ap * F_IN
                                            + (h0 + hi) * D : ap * F_IN
                                            + (h0 + hi + 1) * D,
                                        ],
                                        rhs[hi][:, p2 * 512 : (p2 + 1) * 512],
                                        start=(ap == 0),
                                        stop=(ap == NT - 1),
                                        tile_position=(0, po),
                                        skip_group_check=True,
                                    )
                        if a < NT:
                            prev = prev_new
                    # evacuate the accumulated outT half-panel (both heads);
                    # split across ACT+DVE (different PSUM banks -> parallel)
                    nc.scalar.activation(
                        ot_sb[:, qh * 1024 : qh * 1024 + 512], ot[:, 0:512], COPY
                    )
                    nc.vector.tensor_copy(
                        ot_sb[:, qh * 1024 + 512 : (qh + 1) * 1024], ot[:, 512:1024]
                    )

                # softmax denominators for both heads of the pair
                csl = slice(h0 * NT, h0 * NT + 2 * NT)
                csl1 = slice(64 + h0 * NT, 64 + h0 * NT + 2 * NT)
                nc.vector.tensor_add(rsum[:, csl], rows[:, csl], rows[:, csl1])
                nc.vector.reciprocal(recip[:, csl], rsum[:, csl])

                # finalize: one [128,128] transpose covers both heads' dims;
                # ACT evacuates PSUM (GPSIMD has no PSUM port), GPSIMD does
                # the fused (outT * recip) + h scaled-add off both hot engines
                for qt in range(NT):
                    tr = sa_ps.tile([128, 128], F32, name="tr", tag="sa")
                    nc.tensor.transpose(
                        tr[:],
                        ot_sb[:, qt * 128 : (qt + 1) * 128],
                        ident[:],
                    )
                    o_sb = out_pool.tile([128, 128], F32, name="o_sb", tag="o_sb")
                    for hi in range(2):
                        hh = h0 + hi
                        nc.vector.scalar_tensor_tensor(
                            o_sb[:, hi * D : (hi + 1) * D],
                            tr[:, hi * D : (hi + 1) * D],
                            recip[:, hh * NT + qt : hh * NT + qt + 1],
                            hhi_sb[:, qt * F_IN + hh * D : qt * F_IN + (hh + 1) * D],
                            MULT,
                            ADD,
                        )
                    nc.sync.dma_start(
                        out_d[qt * 128 : (qt + 1) * 128, h0 * D : (h0 + 2) * D],
                        o_sb[:],
                    )


_NC_CACHE = None


def get_nc():
    global _NC_CACHE
    if _NC_CACHE is None:
        _NC_CACHE = _build_program()
    return _NC_CACHE


def make_in_maps(h, W):
    import ml_dtypes

    h = np.ascontiguousarray(np.asarray(h, dtype=np.float32))
    W = np.ascontiguousarray(np.asarray(W, dtype=np.float32))
    h_hi = h.astype(ml_dtypes.bfloat16)
    h_lo = (h - h_hi.astype(np.float32)).astype(ml_dtypes.bfloat16)
    ident = np.eye(128, dtype=np.float32)
    return [
        {"h_hi": h_hi[b], "h_lo": h_lo[b], "w": W, "ident": ident}
        for b in range(N_CORES)
    ]


def run(h, W, trace=False, **kwargs):
    nc = get_nc()
    res = run_bass_kernel_spmd(
        nc, make_in_maps(h, W), core_ids=list(range(N_CORES)), trace=trace, **kwargs
    )
    out = np.stack([res.results[b]["out"] for b in range(N_CORES)], axis=0)
    return out, res


def kernel(h, adj, W):
    out, _ = run(h, W)
    return out


# revision 14
# speedup vs baseline: 1.1692x; 1.1692x over previous
"""Multi-head graph attention layer on 8 Trainium2 NeuronCores.

Reference computation (per batch element b; adj is unused by the reference):
    P      = einsum("nf,hfd->hnd", h[b], W)          # per-head projections
    S      = einsum("hnd,hmd->hnm", P, P)            # scores (symmetric!)
    E      = exp(leakyrelu(S, 0.2) + SHIFT)
    attn   = E / rowsum(E)
    out[b] = concat_heads(attn @ P) + h[b]

Numerical shortcuts (validated against the reference on CPU):
  * leakyrelu is dropped: every row's max score is >= ~24 while lrelu only
    modifies scores < 0, whose softmax weight is < e^-24 relative -- the
    end-to-end deviation is < 1e-6.
  * Half the panels use a Schraudolph-style fast exp on the Vector engine:
    bf16(exp(x)) ~= bitcast_bf16(uint16(round(A*x + B))), A = 128/ln2.
    The f32->uint16 conversion saturates at 0/65535 on HW (probed), which
    clamps the underflow range for free. Max end-to-end rel err ~4e-3.

Sharding: batch B=8 -> one batch element per core (pure data parallel).

Per-core plan (N=2048 tokens, F=256, H=4 heads, D=64):
  Phase 0: hT via PE transposes; P (bf16) and PT (fp16) via f32r matmuls.
  Main loop over head pairs pp, column halves qh, row tiles a:
    PE computes S panels for both heads (row-packed, K=64+64).
    Head 2pp   ("ACT head"): exp on the Scalar engine straight from PSUM,
      bf16 out + fused accum_out giving softmax row-sum partials.
    Head 2pp+1 ("DVE head"): Schraudolph on the Vector engine straight from
      PSUM into uint16 (= bf16 bits), then a 4x-mode tensor_scalar copy over
      the bf16 view with accum_out for its row sums.
    PE accumulates outT[d, q] += P_a^T-block @ E-panel (col-packed heads).
  Finalize per pp: PE-transpose outT chunks; DVE fuses (outT * recip) + h;
  DMA to DRAM.
"""

import math

import numpy as np

import bass_rust
import concourse.bass as bass
import concourse.tile as tile
from concourse import mybir
from concourse.bass_utils import run_bass_kernel_spmd
from concourse.vector_clock import ScopedClock


def _patched_drain_and_barrier(self, tick_clock, wait_clock):
    """Replacement for TileContext._drain_and_barrier.

    The stock version attaches every outstanding semaphore wait (engines +
    every DMA queue used) to ONE tail drain; walrus's setupSyncWait rejects
    instructions with more than a couple of sync waits. Emit a chain of
    drains first, each carrying a single semaphore wait, so the final full
    drain has nothing left to wait on.
    """
    gc = tick_clock.global_clock
    n_procs = 27
    vals = [gc.peek_next(p) - 1 for p in range(n_procs)]
    for p, v in enumerate(vals):
        if v <= 0:
            continue
        partial = bass_rust.VectorClock()
        partial.require_at_least(p, v)
        d = self.nc.sync.drain()
        wait_clock.add_sem_waits(d.ins, ScopedClock({None: partial}))

    # Final drain carries no waits: the chain above already waited out the
    # full global clock on SP, which executes its queue in order.
    self.nc.sync.drain()

    self.nc.all_engine_barrier()
    assert self.sems is not None
    popped = self.nc._tile_sem_poison_stack.pop()
    assert popped is self._sem_poison
    self.nc.clear_and_free_semaphores(list(self.sems.allocated().values()))
    self.nc.all_engine_barrier()


tile.TileContext._drain_and_barrier = _patched_drain_and_barrier


def _split_sync_waits(nc, max_waits=1):
    """walrus's per-instruction sync-wait budget is tiny (LDWEIGHTS rejects
    even 2). Hoist excess waits onto standalone same-engine EventSemaphore
    instructions inserted immediately before the offender — identical
    semantics, one wait per instruction word."""
    n_split = 0
    for f in nc.m.functions:
        for bb in f.blocks:
            il = bb.instructions
            i = 0
            while i < len(il):
                ins = il[i]
                si = ins.sync_info
                waits = list(si.on_wait) if si and si.on_wait else []
                if len(waits) > max_waits:
                    keep = waits[:max_waits]
                    excess = waits[max_waits:]
                    carriers = []
                    for k, w in enumerate(excess):
                        c = bass_rust.InstEventSemaphore(
                            name=f"{ins.name}-w{k}", ins=[], outs=[]
                        )
                        c.engine = ins.engine
                        c.sync_info = mybir.SyncInfo(on_wait=[w], on_update=[])
                        carriers.append(c)
                    ins.sync_info = mybir.SyncInfo(
                        on_wait=keep, on_update=list(si.on_update or [])
                    )
                    il[i:i] = carriers
                    i += len(carriers)
                    n_split += 1
                i += 1
    return n_split


N = 2048
F_IN = 256
H = 4
D = 64
NT = N // 128  # 16 token tiles
N_CORES = 8
# Constant shift inside exp (softmax is shift-invariant). Scores reach
# ~+150 on the diagonal (chi^2_64) which would overflow exp in fp32;
# with C=80 the exp range is [e^-230, e^72] — comfortably finite, and
# row sums stay >= e^(rowmax-80) > 1e-25 so the reciprocal is safe.
EXP_SHIFT = -80.0
# Schraudolph fast-exp in bf16 bit space: u16 = round(A*x + B) saturated
# to [0, 65535]; bitcast to bf16 gives exp(x) with ~2% max elementwise
# error (~4e-3 end-to-end after softmax normalization).
A_SCH = 128.0 / math.log(2.0)
B_SCH = 127.0 * 128.0 - 8.0 + A_SCH * EXP_SHIFT

F32 = mybir.dt.float32
F32R = mybir.dt.float32r
BF16 = mybir.dt.bfloat16
F16 = mybir.dt.float16
U16 = mybir.dt.uint16

# Hoist multi-sem waits into standalone carrier instructions (needed for
# walrus codegen).
SPLIT_WAITS = True
# head1 panels are exp'd by ACT (instead of DVE Schraudolph + reduce) on
# these row-tile iterations — balances the two engines, since a DVE panel
# costs ~2.6us (Schraudolph + 1x-mode row-sum reduce) while an ACT panel
# costs ~1.4us (exp with fused accum).
ACT1_SET = frozenset({1, 4, 7, 10, 13})


def _build_program():
    nc = bass.Bass("TRN2", target_bir_lowering=False, debug=False)
    hhi_d = nc.dram_tensor("h_hi", [N, F_IN], BF16, kind="ExternalInput").ap()
    hlo_d = nc.dram_tensor("h_lo", [N, F_IN], BF16, kind="ExternalInput").ap()
    w_d = nc.dram_tensor("w", [H, F_IN, D], F32, kind="ExternalInput").ap()
    id_d = nc.dram_tensor("ident", [128, 128], F32, kind="ExternalInput").ap()
    out_d = nc.dram_tensor("out", [N, F_IN], F32, kind="ExternalOutput").ap()

    with tile.TileContext(nc) as tc:
        _gat_kernel(tc, out_d, hhi_d, hlo_d, w_d, id_d)
    if SPLIT_WAITS:
        _split_sync_waits(nc)
    return nc


def _gat_kernel(tc: "tile.TileContext", out_d, hhi_d, hlo_d, w_d, id_d):
    nc = tc.nc
    MULT = mybir.AluOpType.mult
    ADD = mybir.AluOpType.add
    EXP = mybir.ActivationFunctionType.Exp
    COPY = mybir.ActivationFunctionType.Copy

    with (
        tc.tile_pool(name="const", bufs=1) as const,
    ):
        # ---------------- persistent SBUF ----------------
        ident = const.tile([128, 128], F32, name="ident_sb")
        nc.sync.dma_start(ident[:], id_d[:])
        shift = const.tile([128, 1], F32, name="shift_sb")
        nc.gpsimd.memset(shift[:], EXP_SHIFT)
        # h tiles (bf16 hi part only — used for the residual add; the lo
        # part is < 0.4% of h and far below the error budget)
        hhi_sb = const.tile([128, NT * F_IN], BF16, name="hhi_sb")  # [p,(qt f)]
        for qt in range(NT):
            nc.sync.dma_start(
                hhi_sb[:, qt * F_IN : (qt + 1) * F_IN],
                hhi_d[qt * 128 : (qt + 1) * 128, :],
            )
        # hT via transposing DMA of the bf16 hi/lo split, then one DVE add
        # to reconstruct fp32(ish) h^T: [p=f, (ft, n)]
        hThi = const.tile([128, 2 * N], BF16, name="hThi")
        hTlo = const.tile([128, 2 * N], BF16, name="hTlo")
        for ft in range(2):
            nc.sync.dma_start_transpose(
                hThi[:, ft * N : (ft + 1) * N], hhi_d[:, ft * 128 : (ft + 1) * 128]
            )
            nc.sync.dma_start_transpose(
                hTlo[:, ft * N : (ft + 1) * N], hlo_d[:, ft * 128 : (ft + 1) * 128]
            )
        w_sb = const.tile([128, 2 * F_IN], F32, name="w_sb")  # [p, (ft, h*64+d)]
        for hh in range(H):
            for ft in range(2):
                nc.sync.dma_start(
                    w_sb[:, ft * F_IN + hh * D : ft * F_IN + (hh + 1) * D],
                    w_d[hh, ft * 128 : (ft + 1) * 128, :],
                )

        w_sbr = const.tile([128, 2 * F_IN], F32R, name="w_sbr")
        nc.vector.tensor_copy(w_sbr[:], w_sb[:])
        hT_sb = const.tile([128, 2 * N], F32R, name="hT_sb")  # [p=f, (ft, n)]
        nc.vector.tensor_add(hT_sb[:], hThi[:], hTlo[:])
        p_bf = const.tile([128, NT * F_IN], BF16, name="p_bf")  # [p=k, (kt, h*64+d)]
        # PT pair tiles: partitions 0-63 = head 2pp dims, 64-127 = head 2pp+1
        pt_sb = [
            const.tile([128, N], F16, name=f"pt_pair{pp}") for pp in range(H // 2)
        ]
        # row-sum partials: col = qh*64 + hh*16 + a  (token tile a, head hh)
        rows = const.tile([128, 2 * H * NT], F32, name="rows")
        rsum = const.tile([128, H * NT], F32, name="rsum")  # col = hh*16 + qt
        recip = const.tile([128, H * NT], F32, name="recip")
        trash = const.tile([128, 1024], BF16, name="trash")

        # ---------------- phase B/C: projections ----------------
        with (
            tc.tile_pool(name="p_ps", bufs=2, space="PSUM") as p_ps,
            tc.tile_pool(name="pt_ps", bufs=2, space="PSUM") as pt_ps,
        ):
            # PT per head-pair first (it gates the S matmuls): fp16
            # [128(d of 2 heads), N(q)]
            for pp in range(H // 2):
                for pan in range(4):
                    ptp = pt_ps.tile([128, 512], F32, name="ptp", tag="ptp")
                    for ft in range(2):
                        nc.tensor.matmul(
                            ptp[:],
                            w_sbr[:, ft * F_IN + pp * 128 : ft * F_IN + (pp + 1) * 128],
                            hT_sb[:, ft * N + pan * 512 : ft * N + (pan + 1) * 512],
                            start=(ft == 0),
                            stop=(ft == 1),
                        )
                    dst = pt_sb[pp][:, pan * 512 : (pan + 1) * 512]
                    if pan % 2 == 0:
                        nc.scalar.activation(dst, ptp[:], COPY)
                    else:
                        nc.vector.tensor_copy(dst, ptp[:])

            # P = h @ W  -> [k, (h d)] tiles, stored bf16
            for i in range(NT):
                pp_t = p_ps.tile([128, F_IN], F32, name="pp", tag="pp")
                for ft in range(2):
                    nc.tensor.matmul(
                        pp_t[:],
                        hT_sb[:, ft * N + i * 128 : ft * N + (i + 1) * 128],
                        w_sbr[:, ft * F_IN : (ft + 1) * F_IN],
                        start=(ft == 0),
                        stop=(ft == 1),
                    )
                dst = p_bf[:, i * F_IN : (i + 1) * F_IN]
                if i % 2 == 0:
                    nc.scalar.activation(dst, pp_t[:], COPY)
                else:
                    nc.vector.tensor_copy(dst, pp_t[:])

        # ---------------- phase D: attention main loop ----------------
        # PSUM: sa pool 2x[128,1024] (4 banks) + sd pool 4x[128,512]
        # (4 banks... wait: 2 bufs = 2 banks) + ot 1x[128,1024] (2 banks).
        with (
            tc.tile_pool(name="sa_ps", bufs=2, space="PSUM") as sa_ps,
            tc.tile_pool(name="sd_ps", bufs=2, space="PSUM") as sd_ps,
            tc.tile_pool(name="ot_ps", bufs=1, space="PSUM") as ot_ps,
            tc.tile_pool(name="e0_pool", bufs=3) as e0_pool,
            tc.tile_pool(name="e1_pool", bufs=3) as e1_pool,
            tc.tile_pool(name="ot_sb_pool", bufs=2) as ot_sb_pool,
            tc.tile_pool(name="out_pool", bufs=6) as out_pool,
        ):

            for pp in range(H // 2):
                h0 = 2 * pp
                h1 = h0 + 1
                ot_sb = ot_sb_pool.tile([128, N], F32, name="ot_sb", tag="ot_sb")
                for qh in range(2):
                    ot = ot_ps.tile([128, 1024], F32, name="ot", tag="ot")
                    # software pipeline: S+exp for tile a, outT for tile a-1
                    prev = None
                    for a in range(NT + 1):
                        if a < NT:
                            head1_act = a in ACT1_SET
                            sa = sa_ps.tile([128, 1024], F32, name="sa", tag="sa")
                            if head1_act:
                                sa1 = sa_ps.tile(
                                    [128, 1024], F32, name="sa1", tag="sa"
                                )
                                sd = None
                            else:
                                sa1 = None
                                sd = [
                                    sd_ps.tile(
                                        [128, 512], F32, name=f"sd{p2}", tag="sd"
                                    )
                                    for p2 in range(2)
                                ]
                            # S matmuls, row-packed head pair (interleave the
                            # two heads per p2 chunk so they stream together)
                            for p2 in range(2):
                                qsl = slice(
                                    qh * 1024 + p2 * 512, qh * 1024 + (p2 + 1) * 512
                                )
                                csl = slice(p2 * 512, (p2 + 1) * 512)
                                nc.tensor.matmul(
                                    sa[:, csl],
                                    pt_sb[pp][0:64, a * 128 : (a + 1) * 128],
                                    pt_sb[pp][0:64, qsl],
                                    start=True,
                                    stop=True,
                                    tile_position=(0, 0),
                                )
                                nc.tensor.matmul(
                                    sa1[:, csl] if head1_act else sd[p2][:],
                                    pt_sb[pp][64:128, a * 128 : (a + 1) * 128],
                                    pt_sb[pp][64:128, qsl],
                                    start=True,
                                    stop=True,
                                    tile_position=(64, 0),
                                )
                            # ACT head: table exp straight from PSUM + accum
                            e0 = e0_pool.tile([128, 1024], BF16, name="e0", tag="e0")
                            acc0 = rows[
                                :, qh * 64 + h0 * NT + a : qh * 64 + h0 * NT + a + 1
                            ]
                            nc.scalar.activation(
                                e0[:], sa[:], EXP, bias=shift[:], accum_out=acc0
                            )
                            # head1: either ACT exp (same as head0) or DVE
                            # Schraudolph into uint16 (bf16 bits) + 1x reduce
                            e1 = e1_pool.tile([128, 1024], U16, name="e1", tag="e1")
                            acc1 = rows[
                                :, qh * 64 + h1 * NT + a : qh * 64 + h1 * NT + a + 1
                            ]
                            if head1_act:
                                nc.scalar.activation(
                                    e1[:].bitcast(BF16),
                                    sa1[:],
                                    EXP,
                                    bias=shift[:],
                                    accum_out=acc1,
                                )
                            else:
                                for p2 in range(2):
                                    nc.vector.tensor_scalar(
                                        e1[:, p2 * 512 : (p2 + 1) * 512],
                                        sd[p2][:],
                                        A_SCH,
                                        B_SCH,
                                        MULT,
                                        ADD,
                                    )
                                nc.vector.tensor_scalar(
                                    trash[:],
                                    e1[:].bitcast(BF16),
                                    1.0,
                                    0.0,
                                    MULT,
                                    ADD,
                                    accum_out=acc1,
                                )
                            prev_new = (e0, e1)
                        if a > 0:
                            e0p, e1p = prev
                            rhs = [e0p[:], e1p[:].bitcast(BF16)]
                            ap = a - 1
                            for p2 in range(2):
                                for hi in range(2):
                                    po = 64 * hi
                                    nc.tensor.matmul(
                                        ot[po : po + 64, p2 * 512 : (p2 + 1) * 512],
                                        p_bf[
                                            :,
                                            ap * F_IN
                                            + (h0 + hi) * D : ap * F_IN
                                            + (h0 + hi + 1) * D,
                                        ],
                                        rhs[hi][:, p2 * 512 : (p2 + 1) * 512],
                                        start=(ap == 0),
                                        stop=(ap == NT - 1),
                                        tile_position=(0, po),
                                        skip_group_check=True,
                                    )
                        if a < NT:
                            prev = prev_new
                    # evacuate the accumulated outT half-panel (both heads);
                    # split across ACT+DVE (different PSUM banks -> parallel)
                    nc.scalar.activation(
                        ot_sb[:, qh * 1024 : qh * 1024 + 512], ot[:, 0:512], COPY
                    )
                    nc.vector.tensor_copy(
                        ot_sb[:, qh * 1024 + 512 : (qh + 1) * 1024], ot[:, 512:1024]
                    )

                # softmax denominators for both heads of the pair
                csl = slice(h0 * NT, h0 * NT + 2 * NT)
                csl1 = slice(64 + h0 * NT, 64 + h0 * NT + 2 * NT)
                nc.vector.tensor_add(rsum[:, csl], rows[:, csl], rows[:, csl1])
                nc.vector.reciprocal(recip[:, csl], rsum[:, csl])

                # finalize: one [128,128] transpose covers both heads' dims;
                # ACT evacuates PSUM (GPSIMD has no PSUM port), GPSIMD does
                # the fused (outT * recip) + h scaled-add off both hot engines
                for qt in range(NT):
                    tr = sa_ps.tile([128, 128], F32, name="tr", tag="sa")
                    nc.tensor.transpose(
                        tr[:],
                        ot_sb[:, qt * 128 : (qt + 1) * 128],
                        ident[:],
                    )
                    o_sb = out_pool.tile([128, 128], F32, name="o_sb", tag="o_sb")
                    for hi in range(2):
                        hh = h0 + hi
                        nc.vector.scalar_tensor_tensor(
                            o_sb[:, hi * D : (hi + 1) * D],
                            tr[:, hi * D : (hi + 1) * D],
                            recip[:, hh * NT + qt : hh * NT + qt + 1],
                            hhi_sb[:, qt * F_IN + hh * D : qt * F_IN + (hh + 1) * D],
                            MULT,
                            ADD,
                        )
                    nc.sync.dma_start(
                        out_d[qt * 128 : (qt + 1) * 128, h0 * D : (h0 + 2) * D],
                        o_sb[:],
                    )


_NC_CACHE = None


def get_nc():
    global _NC_CACHE
    if _NC_CACHE is None:
        _NC_CACHE = _build_program()
    return _NC_CACHE


def make_in_maps(h, W):
    import ml_dtypes

    h = np.ascontiguousarray(np.asarray(h, dtype=np.float32))
    W = np.ascontiguousarray(np.asarray(W, dtype=np.float32))
    h_hi = h.astype(ml_dtypes.bfloat16)
    h_lo = (h - h_hi.astype(np.float32)).astype(ml_dtypes.bfloat16)
    ident = np.eye(128, dtype=np.float32)
    return [
        {"h_hi": h_hi[b], "h_lo": h_lo[b], "w": W, "ident": ident}
        for b in range(N_CORES)
    ]


def run(h, W, trace=False, **kwargs):
    nc = get_nc()
    res = run_bass_kernel_spmd(
        nc, make_in_maps(h, W), core_ids=list(range(N_CORES)), trace=trace, **kwargs
    )
    out = np.stack([res.results[b]["out"] for b in range(N_CORES)], axis=0)
    return out, res


def kernel(h, adj, W):
    out, _ = run(h, W)
    return out
